# revision 9
# baseline (speedup 1.0000x reference)
"""MoE (B=4, S=2048, D=1024, E=8, F=4096, top-2) on 8 Trainium2 NeuronCores.

Strategy (expert-parallel, sparse):
  Stage A (device): token-parallel gate. Each core takes T/8 = 1024 tokens,
    computes logits = x @ gate_w in full fp32 on the PE (exact top-2
    selection), top-2 via the DVE max8 instruction, softmax weights via
    ACT exp + DVE reciprocal. Outputs a dense [tokens, E] combine-weight
    matrix (exact 0 for unselected experts).
  Host dispatch: using the device-computed routing, gather each expert's
    tokens (all-to-all token dispatch done as host data movement).
  Stage B (device): expert-parallel FFN. Core e holds expert e's weights;
    computes gelu(x @ W1 + b1) @ W2 + b2 in bf16 (fp32 accumulation),
    scales by the fp32 combine weight on-device.
  Host combine: scatter-add the per-expert partial outputs (each token is
    hit by exactly 2 experts across cores).

Capacity: per-expert token count is ~2048 (top-2 of 8, T=8192); kernels are
compiled for CAP=2560 columns and automatically rebuilt larger if routing
ever exceeds that.
"""

import numpy as np
import ml_dtypes

import concourse.bacc as bacc
import concourse.mybir as mybir
from concourse.tile import TileContext
from concourse.masks import make_identity
from concourse.bass_utils import run_bass_kernel_spmd

dt = mybir.dt
BF16 = ml_dtypes.bfloat16

B, S, D, E, F, TOP_K = 4, 2048, 1024, 8, 4096, 2
T = B * S                      # 8192 tokens total
TS = T // 8                    # 1024 tokens per core in stage A
P = 128
KD = D // P                    # 8 k-tiles over D
KF = F // P                    # 32 k-tiles over F
NCHUNK = 512                   # moving free dim per matmul (one fp32 PSUM bank)
DEFAULT_CAP = 2208             # per-expert token capacity (4x512 + 160)


def _chunk_plan(cap):
    """Split cap into chunks of <= NCHUNK."""
    out = []
    off = 0
    while off < cap:
        c = min(NCHUNK, cap - off)
        out.append((off, c))
        off += c
    return out

_CACHE = {}


def _build_gate():
    """Stage A: logits (fp32) + top-2 softmax combine weights for TS tokens."""
    nc = bacc.Bacc("TRN2", target_bir_lowering=False, debug=False, num_devices=8)
    xt_d = nc.dram_tensor("xt", [KD, P, TS], dt.float32, kind="ExternalInput")
    gw_d = nc.dram_tensor("gw", [KD, P, E], dt.float32, kind="ExternalInput")
    c_d = nc.dram_tensor("cout", [P, TS // P, E], dt.float32, kind="ExternalOutput")

    NG = TS // P  # 8 groups of 128 tokens

    with TileContext(nc) as tc:
        with (
            tc.tile_pool(name="sb", bufs=1) as sb,
            tc.tile_pool(name="ps", bufs=2, space="PSUM") as ps,
            tc.tile_pool(name="pst", bufs=1, space="PSUM") as pst,
        ):
            xts = sb.tile([P, KD, TS], dt.float32)
            for k in range(KD):
                nc.sync.dma_start(out=xts[:, k, :], in_=xt_d[k])
            gws = sb.tile([P, KD, E], dt.float32)
            for k in range(KD):
                nc.sync.dma_start(out=gws[:, k, :], in_=gw_d[k])

            ident = sb.tile([P, P], dt.float32)
            make_identity(nc, ident[:])

            # logits.T [E, TS] via fp32 matmuls, then copy PSUM -> SBUF.
            # k outer / chunk inner so matmul k only waits for DMA k.
            lg = sb.tile([E, TS], dt.float32)
            nch = TS // NCHUNK
            lgp = []
            for n in range(nch):
                lgp_n = ps.tile([E, NCHUNK], dt.float32, tag=f"lgp{n}")
                lgp.append(lgp_n)
            for k in range(KD):
                for n in range(nch):
                    nc.tensor.matmul(
                        lgp[n][:],
                        lhsT=gws[:, k, :],
                        rhs=xts[:, k, n * NCHUNK:(n + 1) * NCHUNK],
                        start=(k == 0),
                        stop=(k == KD - 1),
                    )
            for n in range(nch):
                nc.scalar.copy(lg[:, n * NCHUNK:(n + 1) * NCHUNK], lgp[n][:])

            # transpose to token-major [128, NG, E]
            lgt_p = pst.tile([P, NG * E], dt.float32)
            for g in range(NG):
                nc.tensor.transpose(
                    out=lgt_p[:, g * E:(g + 1) * E],
                    in_=lg[:, g * P:(g + 1) * P],
                    identity=ident[:E, :E],
                )
            lgt = sb.tile([P, NG, E], dt.float32)
            nc.vector.tensor_copy(lgt[:, :, :].rearrange("p g e -> p (g e)"), lgt_p[:])
            lgt2 = lgt[:, :, :].rearrange("p g e -> p (g e)")

            # top-8 (sorted desc) per token group -> m1, m2
            mx = sb.tile([P, NG * E], dt.float32)
            for g in range(NG):
                nc.vector.max(out=mx[:, g * E:(g + 1) * E], in_=lgt2[:, g * E:(g + 1) * E])
            m1 = mx[:, 0:NG * E:E]          # [P, NG]
            m2 = mx[:, 1:NG * E:E]          # [P, NG]

            # rcp = 1 / (1 + exp(m2 - m1)) per token
            sm = sb.tile([P, NG], dt.float32)
            nc.vector.tensor_tensor(sm[:], m2, m1, op=mybir.AluOpType.subtract)
            e2 = sb.tile([P, NG], dt.float32)
            nc.scalar.activation(e2[:], sm[:], mybir.ActivationFunctionType.Exp)
            dn = sb.tile([P, NG], dt.float32)
            nc.vector.tensor_scalar_add(dn[:], e2[:], 1.0)
            rcp = sb.tile([P, NG], dt.float32)
            nc.vector.reciprocal(rcp[:], dn[:])

            # c = (logit >= m2) * exp(logit - m1) * rcp
            m1b = m1.to_broadcast([P, NG, E])
            m2b = m2.to_broadcast([P, NG, E])
            rcpb = rcp[:, :].to_broadcast([P, NG, E])
            dif = sb.tile([P, NG, E], dt.float32)
            nc.vector.tensor_tensor(dif[:, :, :], lgt[:, :, :], m1b, op=mybir.AluOpType.subtract)
            ew = sb.tile([P, NG, E], dt.float32)
            nc.scalar.activation(
                ew[:, :, :].rearrange("p g e -> p (g e)"),
                dif[:, :, :].rearrange("p g e -> p (g e)"),
                mybir.ActivationFunctionType.Exp,
            )
            sel = sb.tile([P, NG, E], dt.float32)
            nc.vector.tensor_tensor(sel[:, :, :], lgt[:, :, :], m2b, op=mybir.AluOpType.is_ge)
            cw = sb.tile([P, NG, E], dt.float32)
            nc.vector.tensor_tensor(cw[:, :, :], sel[:, :, :], ew[:, :, :], op=mybir.AluOpType.mult)
            nc.vector.tensor_tensor(cw[:, :, :], cw[:, :, :], rcpb, op=mybir.AluOpType.mult)

            nc.sync.dma_start(out=c_d[:], in_=cw[:, :, :])

    nc.compile()
    return nc


def _build_ffn(cap):
    """Stage B: per-expert FFN on `cap` gathered tokens (bf16 matmuls)."""
    chunks = _chunk_plan(cap)
    nc = bacc.Bacc("TRN2", target_bir_lowering=False, debug=False, num_devices=8)
    xt_d = nc.dram_tensor("xt", [KD, P, cap], dt.bfloat16, kind="ExternalInput")
    w1_d = nc.dram_tensor("w1", [KD, P, F], dt.bfloat16, kind="ExternalInput")
    w2_d = nc.dram_tensor("w2", [KF, P, D], dt.bfloat16, kind="ExternalInput")
    b1_d = nc.dram_tensor("b1", [P, KF], dt.float32, kind="ExternalInput")
    b2_d = nc.dram_tensor("b2", [P, KD], dt.float32, kind="ExternalInput")
    cw_d = nc.dram_tensor("cw", [1, cap], dt.float32, kind="ExternalInput")
    yt_d = nc.dram_tensor("yt", [KD, P, cap], dt.float32, kind="ExternalOutput")

    with TileContext(nc) as tc:
        with (
            tc.tile_pool(name="const", bufs=1) as cst,
            tc.tile_pool(name="w1p", bufs=8) as w1p,
            tc.tile_pool(name="xtp", bufs=2) as xtp,
            tc.tile_pool(name="hp", bufs=1) as hpool,
            tc.tile_pool(name="stg", bufs=3) as stg,
            tc.tile_pool(name="psh", bufs=3, space="PSUM") as psh,
            tc.tile_pool(name="pso", bufs=2, space="PSUM") as pso,
        ):
            # resident: W2 (64KB/part), biases, combine weights.
            # Loaded via gpsimd (SWDGE) so the bulk W2 transfer doesn't head-
            # of-line-block the HWDGE queues that feed the first matmuls.
            cws = cst.tile([1, cap], dt.float32)
            nc.gpsimd.dma_start(out=cws[:], in_=cw_d[:])
            b1s = cst.tile([P, KF], dt.float32)
            nc.gpsimd.dma_start(out=b1s[:], in_=b1_d[:])
            b2s = cst.tile([P, KD], dt.float32)
            nc.gpsimd.dma_start(out=b2s[:], in_=b2_d[:])
            w2s = cst.tile([P, KF, D], dt.bfloat16)
            for kf in range(KF):
                nc.gpsimd.dma_start(out=w2s[:, kf, :], in_=w2_d[kf])

            # combine weights broadcast to all 128 partitions (on idle GpSimd)
            cbc = cst.tile([P, cap], dt.float32)
            nc.gpsimd.partition_broadcast(cbc[:], cws[0:1, :])

            for (co, cn) in chunks:
                cs = slice(co, co + cn)
                xt_t = xtp.tile([P, KD, NCHUNK], dt.bfloat16, tag="xt")
                nc.sync.dma_start(
                    out=xt_t[:, :, :cn],
                    in_=xt_d[:, :, cs].rearrange("k p c -> p k c"),
                )
                hbuf = hpool.tile([P, KF, NCHUNK], dt.bfloat16, tag="h")
                # h.T = gelu(W1.T @ x.T + b1), F-tile by F-tile
                for m in range(KF):
                    w1t = w1p.tile([P, KD, P], dt.bfloat16, tag="w1t")
                    nc.sync.dma_start(
                        out=w1t[:, :, :],
                        in_=w1_d[:, :, m * P:(m + 1) * P].rearrange("k p c -> p k c"),
                    )
                    hps = psh.tile([P, cn], dt.float32, tag="hps")
                    for k in range(KD):
                        nc.tensor.matmul(
                            hps[:], lhsT=w1t[:, k, :], rhs=xt_t[:, k, :cn],
                            start=(k == 0), stop=(k == KD - 1),
                        )
                    nc.scalar.activation(
                        hbuf[:, m, :cn], hps[:],
                        mybir.ActivationFunctionType.Gelu, bias=b1s[:, m:m + 1],
                    )
                # y.T = W2.T @ h.T + b2, then scale by combine weight
                for my in range(KD):
                    ops = pso.tile([P, cn], dt.float32, tag="ops")
                    for kf in range(KF):
                        nc.tensor.matmul(
                            ops[:], lhsT=w2s[:, kf, my * P:(my + 1) * P],
                            rhs=hbuf[:, kf, :cn],
                            start=(kf == 0), stop=(kf == KF - 1),
                        )
                    st = stg.tile([P, NCHUNK], dt.float32, tag="st")
                    nc.vector.scalar_tensor_tensor(
                        st[:, :cn], ops[:], b2s[:, my:my + 1], cbc[:, cs],
                        op0=mybir.AluOpType.add, op1=mybir.AluOpType.mult,
                    )
                    nc.sync.dma_start(out=yt_d[my, :, cs], in_=st[:, :cn])

    nc.compile()
    return nc


def _get(name, builder):
    if name not in _CACHE:
        _CACHE[name] = builder()
    return _CACHE[name]


def kernel(x, gate_w, w1, b1, w2, b2):
    x = np.asarray(x, np.float32)
    gate_w = np.asarray(gate_w, np.float32)
    w1 = np.asarray(w1, np.float32)
    b1 = np.asarray(b1, np.float32)
    w2 = np.asarray(w2, np.float32)
    b2 = np.asarray(b2, np.float32)

    xf = x.reshape(T, D)

    # ---- Stage A: gate + routing on device (token-parallel) ----
    nc_a = _get("gate", _build_gate)
    gw_r = np.ascontiguousarray(gate_w.reshape(KD, P, E))
    in_a = []
    for j in range(8):
        shard = xf[j * TS:(j + 1) * TS]                       # [TS, D]
        xt = np.ascontiguousarray(shard.T).reshape(KD, P, TS)  # [k, p, t]
        in_a.append({"xt": xt, "gw": gw_r})
    res_a = run_bass_kernel_spmd(nc_a, in_a, core_ids=list(range(8)))
    # cout [P, NG, E] with token t_local = g*128 + p
    c_all = np.concatenate(
        [r["cout"].transpose(1, 0, 2).reshape(TS, E) for r in res_a.results], axis=0
    )  # [T, E]

    # ---- Host dispatch (all-to-all by device-computed routing) ----
    idx_list = [np.nonzero(c_all[:, e])[0] for e in range(E)]
    max_n = max(len(i) for i in idx_list)
    cap = DEFAULT_CAP
    if max_n > cap:
        cap = int(-(-int(max_n * 1.1) // NCHUNK) * NCHUNK)
    nc_b = _get(f"ffn_{cap}", lambda: _build_ffn(cap))

    in_b = []
    for e in range(E):
        idx = idx_list[e]
        ne = len(idx)
        xsel = np.zeros((cap, D), np.float32)
        xsel[:ne] = xf[idx]
        xt = np.ascontiguousarray(xsel.T.astype(BF16)).reshape(KD, P, cap)
        cwv = np.zeros((1, cap), np.float32)
        cwv[0, :ne] = c_all[idx, e]
        in_b.append({
            "xt": xt,
            "w1": np.ascontiguousarray(w1[e].astype(BF16)).reshape(KD, P, F),
            "w2": np.ascontiguousarray(w2[e].astype(BF16)).reshape(KF, P, D),
            "b1": np.ascontiguousarray(b1[e].reshape(KF, P).T),
            "b2": np.ascontiguousarray(b2[e].reshape(KD, P).T),
            "cw": cwv,
        })
    res_b = run_bass_kernel_spmd(nc_b, in_b, core_ids=list(range(8)))

    # ---- Host combine (scatter-add partial outputs) ----
    out = np.zeros((T, D), np.float32)
    for e in range(E):
        idx = idx_list[e]
        yt = res_b.results[e]["yt"]            # [KD, P, cap]
        y = yt.reshape(D, cap)                 # [d, j]
        out[idx] += y[:, :len(idx)].T
    return out.reshape(B, S, D)


# revision 25
# speedup vs baseline: 1.0680x; 1.0680x over previous
"""MoE (B=4, S=2048, D=1024, E=8, F=4096, top-2) on 8 Trainium2 NeuronCores.

Strategy (expert-parallel, sparse):
  Stage A (device): token-parallel gate. Each core takes T/8 = 1024 tokens,
    computes logits = x @ gate_w in full fp32 on the PE (exact top-2
    selection), top-2 via the DVE max8 instruction, softmax weights via
    ACT exp + DVE reciprocal. Outputs a dense [tokens, E] combine-weight
    matrix (exact 0 for unselected experts).
  Host dispatch: using the device-computed routing, gather each expert's
    tokens (all-to-all token dispatch done as host data movement).
  Stage B (device): expert-parallel FFN. Core e holds expert e's weights;
    computes gelu(x @ W1 + b1) @ W2 + b2 in bf16 (fp32 accumulation),
    scales by the fp32 combine weight on-device.
  Host combine: scatter-add the per-expert partial outputs (each token is
    hit by exactly 2 experts across cores).

Capacity: per-expert token count is ~2048 (top-2 of 8, T=8192); kernels are
compiled for CAP=2560 columns and automatically rebuilt larger if routing
ever exceeds that.
"""

import numpy as np
import ml_dtypes

import concourse.bacc as bacc
import concourse.mybir as mybir
from concourse.tile import TileContext
from concourse.masks import make_identity
from concourse.bass_utils import run_bass_kernel_spmd

dt = mybir.dt
BF16 = ml_dtypes.bfloat16

B, S, D, E, F, TOP_K = 4, 2048, 1024, 8, 4096, 2
T = B * S                      # 8192 tokens total
TS = T // 8                    # 1024 tokens per core in stage A
P = 128
KD = D // P                    # 8 k-tiles over D
KF = F // P                    # 32 k-tiles over F
NCHUNK = 512                   # moving free dim per matmul (one fp32 PSUM bank)
DEFAULT_CAP = 2208             # per-expert token capacity (4x512 + 160)


def _chunk_plan(cap):
    """Split cap into chunks of <= NCHUNK."""
    out = []
    off = 0
    while off < cap:
        c = min(NCHUNK, cap - off)
        out.append((off, c))
        off += c
    return out

_CACHE = {}


def _build_gate():
    """Stage A: logits (fp32) + top-2 softmax combine weights for TS tokens."""
    nc = bacc.Bacc("TRN2", target_bir_lowering=False, debug=False, num_devices=8)
    xt_d = nc.dram_tensor("xt", [KD, P, TS], dt.float32, kind="ExternalInput")
    gw_d = nc.dram_tensor("gw", [KD, P, E], dt.float32, kind="ExternalInput")
    c_d = nc.dram_tensor("cout", [P, TS // P, E], dt.float32, kind="ExternalOutput")

    NG = TS // P  # 8 groups of 128 tokens

    with TileContext(nc) as tc:
        with (
            tc.tile_pool(name="sb", bufs=1) as sb,
            tc.tile_pool(name="ps", bufs=2, space="PSUM") as ps,
            tc.tile_pool(name="pst", bufs=1, space="PSUM") as pst,
        ):
            # PE warmup: >4us of sustained dummy matmuls during the input DMA
            # so the HAM clock gate reaches 2.4 GHz before the fp32 logits.
            wsrc = sb.tile([P, 64], dt.bfloat16)
            nc.vector.memset(wsrc[:], 0.25)
            wps = ps.tile([64, 64], dt.float32, tag="warm")
            for _ in range(110):
                nc.tensor.matmul(wps[:], lhsT=wsrc[:, :64], rhs=wsrc[:, :64],
                                 start=True, stop=True)

            gws = sb.tile([P, KD, E], dt.float32)
            nc.sync.dma_start(out=gws[:, :, :], in_=gw_d[:].rearrange("k p e -> p k e"))
            xts = sb.tile([P, KD, TS], dt.float32)
            for k in range(KD):
                nc.sync.dma_start(out=xts[:, k, :], in_=xt_d[k])

            ident = sb.tile([P, P], dt.float32)
            make_identity(nc, ident[:])

            # logits.T [E, TS] via fp32 matmuls, then copy PSUM -> SBUF.
            # k outer / chunk inner so matmul k only waits for DMA k.
            lg = sb.tile([E, TS], dt.float32)
            nch = TS // NCHUNK
            lgp = []
            for n in range(nch):
                lgp_n = ps.tile([E, NCHUNK], dt.float32, tag=f"lgp{n}")
                lgp.append(lgp_n)
            for k in range(KD):
                for n in range(nch):
                    nc.tensor.matmul(
                        lgp[n][:],
                        lhsT=gws[:, k, :],
                        rhs=xts[:, k, n * NCHUNK:(n + 1) * NCHUNK],
                        start=(k == 0),
                        stop=(k == KD - 1),
                    )
            for n in range(nch):
                nc.scalar.copy(lg[:, n * NCHUNK:(n + 1) * NCHUNK], lgp[n][:])

            # transpose to token-major [128, NG, E]
            lgt_p = pst.tile([P, NG * E], dt.float32)
            for g in range(NG):
                nc.tensor.transpose(
                    out=lgt_p[:, g * E:(g + 1) * E],
                    in_=lg[:, g * P:(g + 1) * P],
                    identity=ident[:E, :E],
                )
            lgt = sb.tile([P, NG, E], dt.float32)
            nc.vector.tensor_copy(lgt[:, :, :].rearrange("p g e -> p (g e)"), lgt_p[:])
            lgt2 = lgt[:, :, :].rearrange("p g e -> p (g e)")

            # top-8 (sorted desc) per token group -> m1, m2
            mx = sb.tile([P, NG * E], dt.float32)
            for g in range(NG):
                nc.vector.max(out=mx[:, g * E:(g + 1) * E], in_=lgt2[:, g * E:(g + 1) * E])
            m1 = mx[:, 0:NG * E:E]          # [P, NG]
            m2 = mx[:, 1:NG * E:E]          # [P, NG]

            # rcp = 1 / (1 + exp(m2 - m1)) per token
            sm = sb.tile([P, NG], dt.float32)
            nc.vector.tensor_tensor(sm[:], m2, m1, op=mybir.AluOpType.subtract)
            e2 = sb.tile([P, NG], dt.float32)
            nc.scalar.activation(e2[:], sm[:], mybir.ActivationFunctionType.Exp)
            dn = sb.tile([P, NG], dt.float32)
            nc.vector.tensor_scalar_add(dn[:], e2[:], 1.0)
            rcp = sb.tile([P, NG], dt.float32)
            nc.vector.reciprocal(rcp[:], dn[:])

            # c = (logit >= m2) * exp(logit - m1) * rcp
            m1b = m1.to_broadcast([P, NG, E])
            m2b = m2.to_broadcast([P, NG, E])
            rcpb = rcp[:, :].to_broadcast([P, NG, E])
            dif = sb.tile([P, NG, E], dt.float32)
            nc.vector.tensor_tensor(dif[:, :, :], lgt[:, :, :], m1b, op=mybir.AluOpType.subtract)
            ew = sb.tile([P, NG, E], dt.float32)
            nc.scalar.activation(
                ew[:, :, :].rearrange("p g e -> p (g e)"),
                dif[:, :, :].rearrange("p g e -> p (g e)"),
                mybir.ActivationFunctionType.Exp,
            )
            sel = sb.tile([P, NG, E], dt.float32)
            nc.vector.tensor_tensor(sel[:, :, :], lgt[:, :, :], m2b, op=mybir.AluOpType.is_ge)
            cw = sb.tile([P, NG, E], dt.float32)
            nc.vector.tensor_tensor(cw[:, :, :], sel[:, :, :], ew[:, :, :], op=mybir.AluOpType.mult)
            nc.vector.tensor_tensor(cw[:, :, :], cw[:, :, :], rcpb, op=mybir.AluOpType.mult)

            nc.sync.dma_start(out=c_d[:], in_=cw[:, :, :])

    nc.compile()
    return nc


def _build_ffn(cap):
    """Stage B: per-expert FFN on `cap` gathered tokens (bf16 matmuls)."""
    chunks = _chunk_plan(cap)
    nc = bacc.Bacc("TRN2", target_bir_lowering=False, debug=False, num_devices=8)
    xt_d = nc.dram_tensor("xt", [KD, P, cap], dt.bfloat16, kind="ExternalInput")
    # w1 host layout [m, p, k, c]: w1_d[m, p, k, c] = w1[128k+p, 128m+c]
    w1_d = nc.dram_tensor("w1", [KF, P, KD, P], dt.bfloat16, kind="ExternalInput")
    w2_d = nc.dram_tensor("w2", [KF, P, D], dt.bfloat16, kind="ExternalInput")
    b1_d = nc.dram_tensor("b1", [P, KF], dt.float32, kind="ExternalInput")
    b2_d = nc.dram_tensor("b2", [P, KD], dt.float32, kind="ExternalInput")
    cw_d = nc.dram_tensor("cw", [1, cap], dt.float32, kind="ExternalInput")
    yt_d = nc.dram_tensor("yt", [KD, P, cap], dt.float32, kind="ExternalOutput")

    with TileContext(nc) as tc:
        with (
            tc.tile_pool(name="const", bufs=1) as cst,
            tc.tile_pool(name="xtp", bufs=2) as xtp,
            tc.tile_pool(name="cwp", bufs=2) as cwp,
            tc.tile_pool(name="hp", bufs=1) as hpool,
            tc.tile_pool(name="stg", bufs=3) as stg,
            tc.tile_pool(name="psh", bufs=3, space="PSUM") as psh,
            tc.tile_pool(name="pso", bufs=3, space="PSUM") as pso,
            tc.tile_pool(name="psw", bufs=1, space="PSUM") as psw,
        ):
            # PE warmup: >4us of sustained dummy matmuls during the input DMA
            # so the HAM clock gate reaches 2.4 GHz before the real stream.
            wsrc = cst.tile([P, 64], dt.bfloat16)
            nc.vector.memset(wsrc[:], 0.25)
            wps = psw.tile([64, 64], dt.float32, tag="warm")
            for _ in range(110):
                nc.tensor.matmul(wps[:], lhsT=wsrc[:, :64], rhs=wsrc[:, :64],
                                 start=True, stop=True)

            # W1 resident, streamed on HWDGE in m order (first matmuls need
            # only the m=0 slice). W2/biases on gpsimd (SWDGE) so the bulk
            # transfer doesn't head-of-line-block the HWDGE queues.
            b1s = cst.tile([P, KF], dt.float32)
            nc.gpsimd.dma_start(out=b1s[:], in_=b1_d[:])
            b2s = cst.tile([P, KD], dt.float32)
            nc.gpsimd.dma_start(out=b2s[:], in_=b2_d[:])
            w1s = cst.tile([P, KF, KD, P], dt.bfloat16)
            w2s = cst.tile([P, KF, D], dt.bfloat16)
            for kf in range(KF):
                nc.gpsimd.dma_start(out=w2s[:, kf, :], in_=w2_d[kf])

            for ci, (co, cn) in enumerate(chunks):
                cs = slice(co, co + cn)
                xt_t = xtp.tile([P, KD, NCHUNK], dt.bfloat16, tag="xt")
                nc.sync.dma_start(
                    out=xt_t[:, :, :cn],
                    in_=xt_d[:, :, cs].rearrange("k p c -> p k c"),
                )
                # combine weights for this chunk, broadcast to all partitions
                cwc = cwp.tile([1, NCHUNK], dt.float32, tag="cwc")
                nc.sync.dma_start(out=cwc[:, :cn], in_=cw_d[:, cs])
                cbcc = cwp.tile([P, NCHUNK], dt.float32, tag="cbcc")
                nc.gpsimd.partition_broadcast(cbcc[:, :cn], cwc[0:1, :cn])

                hbuf = hpool.tile([P, KF, NCHUNK], dt.bfloat16, tag="h")
                # h.T = gelu(W1.T @ x.T + b1), F-tile by F-tile
                for m in range(KF):
                    if ci == 0:
                        # W1 m-slice streamed in just ahead of its matmuls;
                        # stays resident for the remaining chunks.
                        nc.sync.dma_start(out=w1s[:, m, :, :], in_=w1_d[m])
                    hps = psh.tile([P, cn], dt.float32, tag="hps")
                    for k in range(KD):
                        nc.tensor.matmul(
                            hps[:], lhsT=w1s[:, m, k, :],
                            rhs=xt_t[:, k, :cn],
                            start=(k == 0), stop=(k == KD - 1),
                        )
                    nc.scalar.activation(
                        hbuf[:, m, :cn], hps[:],
                        mybir.ActivationFunctionType.Gelu, bias=b1s[:, m:m + 1],
                    )
                # y.T = W2.T @ h.T + b2, then scale by combine weight;
                # epilogue applied in-place in PSUM, DMA out straight from PSUM
                for my in range(KD):
                    ops = pso.tile([P, cn], dt.float32, tag="ops")
                    for kf in range(KF):
                        nc.tensor.matmul(
                            ops[:], lhsT=w2s[:, kf, my * P:(my + 1) * P],
                            rhs=hbuf[:, kf, :cn],
                            start=(kf == 0), stop=(kf == KF - 1),
                        )
                    st = stg.tile([P, NCHUNK], dt.float32, tag="st")
                    nc.vector.scalar_tensor_tensor(
                        st[:, :cn], ops[:], b2s[:, my:my + 1], cbcc[:, :cn],
                        op0=mybir.AluOpType.add, op1=mybir.AluOpType.mult,
                    )
                    nc.sync.dma_start(out=yt_d[my, :, cs], in_=st[:, :cn])

    nc.compile()
    return nc


def _get(name, builder):
    if name not in _CACHE:
        _CACHE[name] = builder()
    return _CACHE[name]


def prep_a_inputs(xf, gate_w):
    gw_r = np.ascontiguousarray(gate_w.reshape(KD, P, E))
    in_a = []
    for j in range(8):
        shard = xf[j * TS:(j + 1) * TS]                       # [TS, D]
        xt = np.ascontiguousarray(shard.T).reshape(KD, P, TS)  # [k, p, t]
        in_a.append({"xt": xt, "gw": gw_r})
    return in_a


def routing_from_a(res_a):
    # cout [P, NG, E] with token t_local = g*128 + p
    return np.concatenate(
        [r["cout"].transpose(1, 0, 2).reshape(TS, E) for r in res_a.results], axis=0
    )  # [T, E]


def prep_b_inputs(xf, w1, b1, w2, b2, c_all, idx_list, cap):
    in_b = []
    for e in range(E):
        idx = idx_list[e]
        ne = len(idx)
        xsel = np.zeros((cap, D), np.float32)
        xsel[:ne] = xf[idx]
        xt = np.ascontiguousarray(xsel.T.astype(BF16)).reshape(KD, P, cap)
        cwv = np.zeros((1, cap), np.float32)
        cwv[0, :ne] = c_all[idx, e]
        in_b.append({
            "xt": xt,
            "w1": np.ascontiguousarray(
                w1[e].astype(BF16).reshape(KD, P, KF, P).transpose(2, 1, 0, 3)),
            "w2": np.ascontiguousarray(w2[e].astype(BF16)).reshape(KF, P, D),
            "b1": np.ascontiguousarray(b1[e].reshape(KF, P).T),
            "b2": np.ascontiguousarray(b2[e].reshape(KD, P).T),
            "cw": cwv,
        })
    return in_b


def kernel(x, gate_w, w1, b1, w2, b2):
    x = np.asarray(x, np.float32)
    gate_w = np.asarray(gate_w, np.float32)
    w1 = np.asarray(w1, np.float32)
    b1 = np.asarray(b1, np.float32)
    w2 = np.asarray(w2, np.float32)
    b2 = np.asarray(b2, np.float32)

    xf = x.reshape(T, D)

    # ---- Stage A: gate + routing on device (token-parallel) ----
    nc_a = _get("gate", _build_gate)
    in_a = prep_a_inputs(xf, gate_w)
    res_a = run_bass_kernel_spmd(nc_a, in_a, core_ids=list(range(8)))
    c_all = routing_from_a(res_a)

    # ---- Host dispatch (all-to-all by device-computed routing) ----
    idx_list = [np.nonzero(c_all[:, e])[0] for e in range(E)]
    max_n = max(len(i) for i in idx_list)
    cap = DEFAULT_CAP
    if max_n > cap:
        cap = int(-(-int(max_n * 1.1) // NCHUNK) * NCHUNK)
    nc_b = _get(f"ffn_{cap}", lambda: _build_ffn(cap))

    in_b = prep_b_inputs(xf, w1, b1, w2, b2, c_all, idx_list, cap)
    res_b = run_bass_kernel_spmd(nc_b, in_b, core_ids=list(range(8)))

    # ---- Host combine (scatter-add partial outputs) ----
    out = np.zeros((T, D), np.float32)
    for e in range(E):
        idx = idx_list[e]
        yt = res_b.results[e]["yt"]            # [KD, P, cap]
        y = yt.reshape(D, cap)                 # [d, j]
        out[idx] += y[:, :len(idx)].T
    return out.reshape(B, S, D)


# revision 29
# speedup vs baseline: 1.0822x; 1.0133x over previous
"""MoE (B=4, S=2048, D=1024, E=8, F=4096, top-2) on 8 Trainium2 NeuronCores.

Strategy (expert-parallel, sparse):
  Stage A (device): token-parallel gate. Each core takes T/8 = 1024 tokens,
    computes logits = x @ gate_w in full fp32 on the PE (exact top-2
    selection), top-2 via the DVE max8 instruction, softmax weights via
    ACT exp + DVE reciprocal. Outputs a dense [tokens, E] combine-weight
    matrix (exact 0 for unselected experts).
  Host dispatch: using the device-computed routing, gather each expert's
    tokens (all-to-all token dispatch done as host data movement).
  Stage B (device): expert-parallel FFN. Core e holds expert e's weights;
    computes gelu(x @ W1 + b1) @ W2 + b2 in bf16 (fp32 accumulation),
    scales by the fp32 combine weight on-device.
  Host combine: scatter-add the per-expert partial outputs (each token is
    hit by exactly 2 experts across cores).

Capacity: per-expert token count is ~2048 (top-2 of 8, T=8192); kernels are
compiled for CAP=2208 columns and automatically rebuilt larger if routing
ever exceeds that.
"""

import numpy as np
import ml_dtypes

import concourse.bacc as bacc
import concourse.mybir as mybir
from concourse.tile import TileContext
from concourse.masks import make_identity
from concourse.bass_utils import run_bass_kernel_spmd

dt = mybir.dt
BF16 = ml_dtypes.bfloat16

B, S, D, E, F, TOP_K = 4, 2048, 1024, 8, 4096, 2
T = B * S                      # 8192 tokens total
TS = T // 8                    # 1024 tokens per core in stage A
P = 128
KD = D // P                    # 8 k-tiles over D
KF = F // P                    # 32 k-tiles over F
NCHUNK = 512                   # moving free dim per matmul (one fp32 PSUM bank)
DEFAULT_CAP = 2208             # per-expert token capacity (4x512 + 160)


def _chunk_plan(cap):
    """Split cap into chunks of <= NCHUNK."""
    out = []
    off = 0
    while off < cap:
        c = min(NCHUNK, cap - off)
        out.append((off, c))
        off += c
    return out

_CACHE = {}


def _build_gate():
    """Stage A: logits (fp32) + top-2 softmax combine weights for TS tokens."""
    nc = bacc.Bacc("TRN2", target_bir_lowering=False, debug=False, num_devices=8)
    xt_d = nc.dram_tensor("xt", [KD, P, TS], dt.float32, kind="ExternalInput")
    gw_d = nc.dram_tensor("gw", [KD, P, E], dt.float32, kind="ExternalInput")
    c_d = nc.dram_tensor("cout", [P, TS // P, E], dt.float32, kind="ExternalOutput")

    NG = TS // P  # 8 groups of 128 tokens

    with TileContext(nc) as tc:
        with (
            tc.tile_pool(name="sb", bufs=1) as sb,
            tc.tile_pool(name="ps", bufs=2, space="PSUM") as ps,
            tc.tile_pool(name="pst", bufs=1, space="PSUM") as pst,
        ):
            # PE warmup: >4us of sustained dummy matmuls during the input DMA
            # so the HAM clock gate reaches 2.4 GHz before the fp32 logits.
            wsrc = sb.tile([P, 64], dt.bfloat16)
            nc.vector.memset(wsrc[:], 0.25)
            wps = ps.tile([64, 64], dt.float32, tag="warm")
            for _ in range(110):
                nc.tensor.matmul(wps[:], lhsT=wsrc[:, :64], rhs=wsrc[:, :64],
                                 start=True, stop=True)

            gws = sb.tile([P, KD, E], dt.float32)
            nc.sync.dma_start(out=gws[:, :, :], in_=gw_d[:].rearrange("k p e -> p k e"))
            xts = sb.tile([P, KD, TS], dt.float32)
            for k in range(KD):
                nc.sync.dma_start(out=xts[:, k, :], in_=xt_d[k])

            ident = sb.tile([P, P], dt.float32)
            make_identity(nc, ident[:])

            # logits.T [E, TS] via fp32 matmuls, then copy PSUM -> SBUF.
            # k outer / chunk inner so matmul k only waits for DMA k.
            lg = sb.tile([E, TS], dt.float32)
            nch = TS // NCHUNK
            lgp = []
            for n in range(nch):
                lgp_n = ps.tile([E, NCHUNK], dt.float32, tag=f"lgp{n}")
                lgp.append(lgp_n)
            for k in range(KD):
                for n in range(nch):
                    nc.tensor.matmul(
                        lgp[n][:],
                        lhsT=gws[:, k, :],
                        rhs=xts[:, k, n * NCHUNK:(n + 1) * NCHUNK],
                        start=(k == 0),
                        stop=(k == KD - 1),
                    )
            for n in range(nch):
                nc.scalar.copy(lg[:, n * NCHUNK:(n + 1) * NCHUNK], lgp[n][:])

            # transpose to token-major [128, NG, E]
            lgt_p = pst.tile([P, NG * E], dt.float32)
            for g in range(NG):
                nc.tensor.transpose(
                    out=lgt_p[:, g * E:(g + 1) * E],
                    in_=lg[:, g * P:(g + 1) * P],
                    identity=ident[:E, :E],
                )
            lgt = sb.tile([P, NG, E], dt.float32)
            nc.vector.tensor_copy(lgt[:, :, :].rearrange("p g e -> p (g e)"), lgt_p[:])
            lgt2 = lgt[:, :, :].rearrange("p g e -> p (g e)")

            # top-8 (sorted desc) per token group -> m1, m2
            mx = sb.tile([P, NG * E], dt.float32)
            for g in range(NG):
                nc.vector.max(out=mx[:, g * E:(g + 1) * E], in_=lgt2[:, g * E:(g + 1) * E])
            m1 = mx[:, 0:NG * E:E]          # [P, NG]
            m2 = mx[:, 1:NG * E:E]          # [P, NG]

            # rcp = 1 / (1 + exp(m2 - m1)) per token
            sm = sb.tile([P, NG], dt.float32)
            nc.vector.tensor_tensor(sm[:], m2, m1, op=mybir.AluOpType.subtract)
            e2 = sb.tile([P, NG], dt.float32)
            nc.scalar.activation(e2[:], sm[:], mybir.ActivationFunctionType.Exp)
            dn = sb.tile([P, NG], dt.float32)
            nc.vector.tensor_scalar_add(dn[:], e2[:], 1.0)
            rcp = sb.tile([P, NG], dt.float32)
            nc.vector.reciprocal(rcp[:], dn[:])

            # c = (logit >= m2) * exp(logit - m1) * rcp
            m1b = m1.to_broadcast([P, NG, E])
            m2b = m2.to_broadcast([P, NG, E])
            rcpb = rcp[:, :].to_broadcast([P, NG, E])
            dif = sb.tile([P, NG, E], dt.float32)
            nc.vector.tensor_tensor(dif[:, :, :], lgt[:, :, :], m1b, op=mybir.AluOpType.subtract)
            ew = sb.tile([P, NG, E], dt.float32)
            nc.scalar.activation(
                ew[:, :, :].rearrange("p g e -> p (g e)"),
                dif[:, :, :].rearrange("p g e -> p (g e)"),
                mybir.ActivationFunctionType.Exp,
            )
            sel = sb.tile([P, NG, E], dt.float32)
            nc.vector.tensor_tensor(sel[:, :, :], lgt[:, :, :], m2b, op=mybir.AluOpType.is_ge)
            cw = sb.tile([P, NG, E], dt.float32)
            nc.vector.tensor_tensor(cw[:, :, :], sel[:, :, :], ew[:, :, :], op=mybir.AluOpType.mult)
            nc.vector.tensor_tensor(cw[:, :, :], cw[:, :, :], rcpb, op=mybir.AluOpType.mult)

            nc.sync.dma_start(out=c_d[:], in_=cw[:, :, :])

    nc.compile()
    return nc


def _build_ffn(cap):
    """Stage B: per-expert FFN on `cap` gathered tokens (bf16 matmuls)."""
    chunks = _chunk_plan(cap)
    nc = bacc.Bacc("TRN2", target_bir_lowering=False, debug=False, num_devices=8)
    xt_d = nc.dram_tensor("xt", [KD, P, cap], dt.bfloat16, kind="ExternalInput")
    # w1 host layout [m, p, k, c]: w1_d[m, p, k, c] = w1[128k+p, 128m+c]
    w1_d = nc.dram_tensor("w1", [KF, P, KD, P], dt.bfloat16, kind="ExternalInput")
    w2_d = nc.dram_tensor("w2", [KF, P, D], dt.bfloat16, kind="ExternalInput")
    b1_d = nc.dram_tensor("b1", [P, KF], dt.float32, kind="ExternalInput")
    b2_d = nc.dram_tensor("b2", [P, KD], dt.float32, kind="ExternalInput")
    cw_d = nc.dram_tensor("cw", [1, cap], dt.float32, kind="ExternalInput")
    yt_d = nc.dram_tensor("yt", [KD, P, cap], dt.float32, kind="ExternalOutput")

    with TileContext(nc) as tc:
        with (
            tc.tile_pool(name="const", bufs=1) as cst,
            tc.tile_pool(name="xtp", bufs=2) as xtp,
            tc.tile_pool(name="cwp", bufs=2) as cwp,
            tc.tile_pool(name="hp", bufs=1) as hpool,
            tc.tile_pool(name="stg", bufs=3) as stg,
            tc.tile_pool(name="psh", bufs=3, space="PSUM") as psh,
            tc.tile_pool(name="pso", bufs=3, space="PSUM") as pso,
            tc.tile_pool(name="psw", bufs=1, space="PSUM") as psw,
        ):
            # PE warmup: >4us of sustained dummy matmuls during the input DMA
            # so the HAM clock gate reaches 2.4 GHz before the real stream.
            wsrc = cst.tile([P, 64], dt.bfloat16)
            nc.vector.memset(wsrc[:], 0.25)
            wps = psw.tile([64, 64], dt.float32, tag="warm")
            for _ in range(110):
                nc.tensor.matmul(wps[:], lhsT=wsrc[:, :64], rhs=wsrc[:, :64],
                                 start=True, stop=True)

            # W1 resident, streamed on HWDGE in m order (first matmuls need
            # only the m=0 slice). W2/biases on gpsimd (SWDGE) so the bulk
            # transfer doesn't head-of-line-block the HWDGE queues.
            b1s = cst.tile([P, KF], dt.float32)
            nc.gpsimd.dma_start(out=b1s[:], in_=b1_d[:])
            b2s = cst.tile([P, KD], dt.float32)
            nc.gpsimd.dma_start(out=b2s[:], in_=b2_d[:])
            w1s = cst.tile([P, KF, KD, P], dt.bfloat16)
            w2s = cst.tile([P, KF, D], dt.bfloat16)
            for kf in range(KF):
                nc.gpsimd.dma_start(out=w2s[:, kf, :], in_=w2_d[kf])

            for ci, (co, cn) in enumerate(chunks):
                cs = slice(co, co + cn)
                xt_t = xtp.tile([P, KD, NCHUNK], dt.bfloat16, tag="xt")
                nc.sync.dma_start(
                    out=xt_t[:, :, :cn],
                    in_=xt_d[:, :, cs].rearrange("k p c -> p k c"),
                )
                # combine weights for this chunk, broadcast to all partitions
                cwc = cwp.tile([1, NCHUNK], dt.float32, tag="cwc")
                nc.sync.dma_start(out=cwc[:, :cn], in_=cw_d[:, cs])
                cbcc = cwp.tile([P, NCHUNK], dt.float32, tag="cbcc")
                nc.gpsimd.partition_broadcast(cbcc[:, :cn], cwc[0:1, :cn])

                hbuf = hpool.tile([P, KF, NCHUNK], dt.bfloat16, tag="h")
                # h.T = gelu(W1.T @ x.T + b1), F-tile by F-tile
                for m in range(KF):
                    if ci == 0:
                        # W1 m-slice streamed in just ahead of its matmuls;
                        # stays resident for the remaining chunks.
                        nc.sync.dma_start(out=w1s[:, m, :, :], in_=w1_d[m])
                    hps = psh.tile([P, cn], dt.float32, tag="hps")
                    for k in range(KD):
                        nc.tensor.matmul(
                            hps[:], lhsT=w1s[:, m, k, :],
                            rhs=xt_t[:, k, :cn],
                            start=(k == 0), stop=(k == KD - 1),
                        )
                    nc.scalar.activation(
                        hbuf[:, m, :cn], hps[:],
                        mybir.ActivationFunctionType.Gelu, bias=b1s[:, m:m + 1],
                    )
                # y.T = W2.T @ h.T + b2, then scale by combine weight
                for my in range(KD):
                    ops = pso.tile([P, cn], dt.float32, tag="ops")
                    for kf in range(KF):
                        nc.tensor.matmul(
                            ops[:], lhsT=w2s[:, kf, my * P:(my + 1) * P],
                            rhs=hbuf[:, kf, :cn],
                            start=(kf == 0), stop=(kf == KF - 1),
                        )
                    st = stg.tile([P, NCHUNK], dt.float32, tag="st")
                    nc.vector.scalar_tensor_tensor(
                        st[:, :cn], ops[:], b2s[:, my:my + 1], cbcc[:, :cn],
                        op0=mybir.AluOpType.add, op1=mybir.AluOpType.mult,
                    )
                    nc.sync.dma_start(out=yt_d[my, :, cs], in_=st[:, :cn])

    nc.compile()
    return nc


def _get(name, builder):
    if name not in _CACHE:
        _CACHE[name] = builder()
    return _CACHE[name]


def prep_a_inputs(xf, gate_w):
    gw_r = np.ascontiguousarray(gate_w.reshape(KD, P, E))
    in_a = []
    for j in range(8):
        shard = xf[j * TS:(j + 1) * TS]                       # [TS, D]
        xt = np.ascontiguousarray(shard.T).reshape(KD, P, TS)  # [k, p, t]
        in_a.append({"xt": xt, "gw": gw_r})
    return in_a


def routing_from_a(res_a):
    # cout [P, NG, E] with token t_local = g*128 + p
    return np.concatenate(
        [r["cout"].transpose(1, 0, 2).reshape(TS, E) for r in res_a.results], axis=0
    )  # [T, E]


def prep_b_inputs(xf, w1, b1, w2, b2, c_all, idx_list, cap):
    in_b = []
    for e in range(E):
        idx = idx_list[e]
        ne = len(idx)
        xsel = np.zeros((cap, D), np.float32)
        xsel[:ne] = xf[idx]
        xt = np.ascontiguousarray(xsel.T.astype(BF16)).reshape(KD, P, cap)
        cwv = np.zeros((1, cap), np.float32)
        cwv[0, :ne] = c_all[idx, e]
        in_b.append({
            "xt": xt,
            "w1": np.ascontiguousarray(
                w1[e].astype(BF16).reshape(KD, P, KF, P).transpose(2, 1, 0, 3)),
            "w2": np.ascontiguousarray(w2[e].astype(BF16)).reshape(KF, P, D),
            "b1": np.ascontiguousarray(b1[e].reshape(KF, P).T),
            "b2": np.ascontiguousarray(b2[e].reshape(KD, P).T),
            "cw": cwv,
        })
    return in_b


def kernel(x, gate_w, w1, b1, w2, b2):
    x = np.asarray(x, np.float32)
    gate_w = np.asarray(gate_w, np.float32)
    w1 = np.asarray(w1, np.float32)
    b1 = np.asarray(b1, np.float32)
    w2 = np.asarray(w2, np.float32)
    b2 = np.asarray(b2, np.float32)

    xf = x.reshape(T, D)

    # ---- Stage A: gate + routing on device (token-parallel) ----
    nc_a = _get("gate", _build_gate)
    in_a = prep_a_inputs(xf, gate_w)
    res_a = run_bass_kernel_spmd(nc_a, in_a, core_ids=list(range(8)))
    c_all = routing_from_a(res_a)

    # ---- Host dispatch (all-to-all by device-computed routing) ----
    idx_list = [np.nonzero(c_all[:, e])[0] for e in range(E)]
    max_n = max(len(i) for i in idx_list)
    cap = DEFAULT_CAP
    if max_n > cap:
        cap = int(-(-int(max_n * 1.1) // NCHUNK) * NCHUNK)
    nc_b = _get(f"ffn_{cap}", lambda: _build_ffn(cap))

    in_b = prep_b_inputs(xf, w1, b1, w2, b2, c_all, idx_list, cap)
    res_b = run_bass_kernel_spmd(nc_b, in_b, core_ids=list(range(8)))

    # ---- Host combine (scatter-add partial outputs) ----
    out = np.zeros((T, D), np.float32)
    for e in range(E):
        idx = idx_list[e]
        yt = res_b.results[e]["yt"]            # [KD, P, cap]
        y = yt.reshape(D, cap)                 # [d, j]
        out[idx] += y[:, :len(idx)].T
    return out.reshape(B, S, D)
